# revision 56
# baseline (speedup 1.0000x reference)
"""Multi-head attention + residual + LayerNorm kernel for 8 TRN2 NeuronCores.

Reference computation (B=2, S=2048, DIM=1024, H=16, DH=64):
    q = x @ Wq.T + bq ; k = x @ Wk.T + bk ; v = x @ Wv.T + bv     (per batch)
    attn_h = softmax((q_h @ k_h.T) / sqrt(DH))
    z_init[b,h,s,d] = attn_h @ v_h
    z = z_init.reshape(B, S, H*DH)      # "faithful" reshape WITHOUT the
                                        # [B,H,S,DH]->[B,S,H,DH] transpose!
    out = LayerNorm(x + z) * gamma + beta

Sharding: core c owns batch c//4 and heads 4r..4r+3 (r = c%4), i.e. output
rows 512r..512r+512.  No collectives; host does transposes/slices/concat.

Schedule: the attention inner loop's softmax exp is split between the
ACT engine (hardware exp, ~1.15us per [128,1024] tile) and the DVE (a
one-instruction Schraudolph bit-trick exp at ~1.22us: int16(s*A+B)
bitcast as bf16; its ~3% element error mostly cancels under softmax
normalization -- measured end-to-end ~8e-3 vs the 2e-2 gate).  With the
DVE taking every 4th tile, the loop paces at the PE's ~1.2us/iteration
(scores pair row-tiled via tile_position, concurrent; PV 2x512; two
woven background matmuls), not ACT's 1.15us -- so everything else is
*woven into* the loop:
  - all attention matmul operands are bf16 (q/k/v/et): f32r scores ran
    at half PE rate (fp32_mode=HIGH, 490ns vs 215ns per 512 cols).
    fp8 DoubleRow projections were measured numerically out of budget
    (2.5e-2 with fp8 q/k).
  - ~32 dummy matmuls on a zeroed tile warm the HAM clock gate during
    the input-DMA window, so the projection chains run at 2.4 GHz.
  - pair 1's projections weave (2 per tb) inside pair 0's attention;
    each pair's second-half qT (a MOVING operand) weaves into its own
    attention.  k/v chains must NOT weave into their own pair's
    attention: the PE's 64-deep reorder window pulls the attention
    LDWEIGHTS (stationary kT/v_sb reads) ahead of in-flight matmuls,
    racing the DVE writes (nondeterministic NaN on hardware).
  - pair 0's scrambled-z transposes (bf16, psum-padded to 66 cols for
    4B alignment) + LayerNorm weave inside pair 1's attention; pair 1's
    run in the tail with hl1's transposes (PE) interleaved against
    hl0's LN chain (DVE).
  - LayerNorm: residual add + stats + normalize all in bf16 (2x/4x DVE
    modes); 1/sqrt(var+eps) via quake bit-trick + 2 Newton steps on a
    batched [128,n] tile, keeping Sqrt off ACT so the exp table set
    never reloads (~2.7us).  gamma/beta passes are compiled out when
    the inputs are the trivial 1/0 (host-checked).
  - output is bf16 (cast to f32 on host); each LN block's store splits
    across the two HWDGE queues (sync + scalar).
Per-core dataflow: scores are computed transposed sT[t,s]=k.q so
softmax's reduction lands on the PV contraction (ones column appended
to v accumulates the denominator), zT blocks are PE-transposed into the
scrambled layout, LN runs rows-on-partitions.
"""

import itertools

import numpy as np
import ml_dtypes

import concourse.bass as bass
import concourse.bacc as bacc
import concourse.mybir as mybir
import concourse.tile as tile
from concourse import bass_utils
from concourse.masks import make_identity

F32 = mybir.dt.float32
F32R = mybir.dt.float32r
BF16 = mybir.dt.bfloat16
I32 = mybir.dt.int32

I16 = mybir.dt.int16

# Schraudolph exp on the DVE: exp(s/8) ~= bitcast_bf16(int16(s * SCHRA_A +
# SCHRA_B)).  A = 0.125 * 2^7/ln2 (score scale folded in); B = 127*2^7 - C
# with C tuned for min max-rel-error (~3%, which softmax normalization
# mostly cancels -- measured end-to-end rel err ~4e-3 at 100% schraudolph).
# One DVE tensor_scalar (f32 PSUM read, int16 convert-on-write) per tile;
# the result bitcasts to a bf16 exp estimate consumed by the PV matmul.
SCHRA_A = 16.0 / 0.6931471805599453
SCHRA_B = 16249.0

B, S, DIM, H = 2, 2048, 1024, 16
DH = DIM // H  # 64
N_CORES = 8
CPB = N_CORES // B   # cores per batch = 4
HPC = H // CPB       # heads per core = 4
SS = S // CPB        # output rows per core = 512
LN_EPS = 1e-5


def build_mha(nc: bass.Bass, trivial_affine: bool = True):
    s_keys, dim, hpc, dh = S, DIM, HPC, DH
    n_pairs = hpc // 2       # head pairs per core (2)
    kc = dim // 128          # contraction chunks (8)
    tb_n = s_keys // 128     # key blocks (16)
    sc_n = s_keys // 512     # 512-wide query chunks (4)
    mrows = s_keys // 16     # scrambled rows per head (128)
    ss_out = hpc * mrows     # output rows per core (512)
    wcols = hpc * dh         # per-core projection output cols (256)

    xT = nc.dram_tensor("xT", [dim, s_keys], BF16, kind="ExternalInput").ap()
    xrow = nc.dram_tensor("xrow", [ss_out, dim], BF16, kind="ExternalInput").ap()
    WqTs = nc.dram_tensor("WqTs", [dim, wcols], BF16, kind="ExternalInput").ap()
    WkTs = nc.dram_tensor("WkTs", [dim, wcols], BF16, kind="ExternalInput").ap()
    WvTs = nc.dram_tensor("WvTs", [dim, wcols], BF16, kind="ExternalInput").ap()
    bqs = nc.dram_tensor("bqs", [wcols], F32, kind="ExternalInput").ap()
    bks = nc.dram_tensor("bks", [wcols], F32, kind="ExternalInput").ap()
    bvs = nc.dram_tensor("bvs", [wcols], F32, kind="ExternalInput").ap()
    gamma = nc.dram_tensor("gamma", [dim], F32, kind="ExternalInput").ap()
    beta = nc.dram_tensor("beta", [dim], F32, kind="ExternalInput").ap()
    out = nc.dram_tensor("out", [ss_out, dim], BF16, kind="ExternalOutput").ap()

    EXP = mybir.ActivationFunctionType.Exp

    with tile.TileContext(nc) as tc:
        with tc.tile_pool(name="singles", bufs=1) as singles, \
             tc.tile_pool(name="kpool", bufs=2) as kpool, \
             tc.tile_pool(name="qpool", bufs=2) as qpool, \
             tc.tile_pool(name="vstage", bufs=2) as vstage_pool, \
             tc.tile_pool(name="vpool", bufs=2) as vpool, \
             tc.tile_pool(name="epool", bufs=4) as epool, \
             tc.tile_pool(name="ztpool", bufs=4) as ztpool, \
             tc.tile_pool(name="rpool", bufs=4) as rpool, \
             tc.tile_pool(name="lnx", bufs=1) as lnx, \
             tc.tile_pool(name="lnw", bufs=2) as lnw, \
             tc.tile_pool(name="ps_pt", bufs=2, space="PSUM") as ps_pt, \
             tc.tile_pool(name="ps_sT", bufs=2, space="PSUM") as ps_sT, \
             tc.tile_pool(name="ps_z", bufs=2, space="PSUM") as ps_z:

            # ---- persistent tiles -------------------------------------------
            xT_sb = singles.tile([128, kc, s_keys], BF16)
            z_all = singles.tile([mrows, hpc, dim], BF16)
            ident = singles.tile([128, 128], F32)
            make_identity(nc, ident)
            ident_bf = singles.tile([128, 128], BF16)
            make_identity(nc, ident_bf)
            ones_col = singles.tile([128, 1], F32)
            nc.vector.memset(ones_col, 1.0)
            # HAM warmup fodder: ~32 dummy matmuls on a zeroed tile keep the
            # PE busy through the input-DMA window, so the HAM clock gate is
            # already 8/8 when the first real projection chain issues (cold
            # chains at 1.2 GHz cost ~10us otherwise).
            warm = singles.tile([128, 512], BF16)
            nc.gpsimd.memset(warm, 0.0)
            for _ in range(24):
                wps = ps_pt.tile([128, 512], F32, tag="pt", name="warmps")
                nc.tensor.matmul(wps, warm[:, 0:128], warm,
                                 start=True, stop=True)
            wq_all = singles.tile([128, kc, wcols], BF16)
            wk_all = singles.tile([128, kc, wcols], BF16)
            wv_all = singles.tile([128, kc, wcols], BF16)
            biases = singles.tile([128, 3, n_pairs], F32)
            grep = singles.tile([128, dim], F32)
            brep = singles.tile([128, dim], F32)

            # ---- DMAs in need-by order --------------------------------------
            # weights first (gate the first projection chains), full-width
            # rows (512B lines), then xT by seq-chunk (kT consumes it
            # t-chunk by t-chunk).
            def _w_dma(w_sb, wt, ch):
                cs = slice(ch * (kc // 2), (ch + 1) * (kc // 2))
                nc.sync.dma_start(
                    out=w_sb[:, cs, :],
                    in_=bass.AP(tensor=wt.tensor,
                                offset=ch * (kc // 2) * 128 * wcols,
                                ap=[[wcols, 128], [128 * wcols, kc // 2],
                                    [1, wcols]]))

            def _x_dma(t, ch):
                cs = slice(ch * (kc // 2), (ch + 1) * (kc // 2))
                nc.sync.dma_start(
                    out=xT_sb[:, cs, t * 512:(t + 1) * 512],
                    in_=bass.AP(
                        tensor=xT.tensor,
                        offset=ch * (kc // 2) * 128 * s_keys + t * 512,
                        ap=[[s_keys, 128], [128 * s_keys, kc // 2], [1, 512]]))

            # need-by order: the k chain is gated on wk + xT chunks 0-1, so
            # those ship first; wq only gates the q chain that follows it.
            for ch in range(2):
                _w_dma(wk_all, WkTs, ch)
            for t in range(2):
                for ch in range(2):
                    _x_dma(t, ch)
            for ch in range(2):
                _w_dma(wq_all, WqTs, ch)
            for ch in range(2):
                _w_dma(wv_all, WvTs, ch)
            for t in range(2, sc_n):
                for ch in range(2):
                    _x_dma(t, ch)
            for j, bt in enumerate((bqs, bks, bvs)):
                nc.scalar.dma_start(
                    out=biases[:, j, :],
                    in_=bass.AP(tensor=bt.tensor, offset=0,
                                ap=[[1, 128], [128, n_pairs]]))
            if not trivial_affine:
                nc.scalar.dma_start(
                    out=grep, in_=bass.AP(tensor=gamma.tensor, offset=0,
                                          ap=[[0, 128], [1, dim]]))
                nc.scalar.dma_start(
                    out=brep, in_=bass.AP(tensor=beta.tensor, offset=0,
                                          ap=[[0, 128], [1, dim]]))
            # residual rows prefetch (read only by LN, which runs woven)
            xt4 = lnx.tile([mrows, hpc, dim], BF16, tag="xt4")
            for sb2 in range(2):
                nc.sync.dma_start(
                    out=xt4[:, 2 * sb2:2 * sb2 + 2, :],
                    in_=bass.AP(tensor=xrow.tensor,
                                offset=2 * sb2 * mrows * dim,
                                ap=[[dim, 128], [mrows * dim, 2], [1, dim]]))

            P = {}   # pl -> dict(qT=, kT=, v=)
            ZT = {}  # (pl, hl) -> zT accumulator tile

            def _alloc_pair(pl):
                if pl not in P:
                    P[pl] = dict(
                        kT=kpool.tile([128, s_keys], BF16, tag="kT",
                                      name=f"kT{pl}"),
                        qT=qpool.tile([128, s_keys], BF16, tag="qT",
                                      name=f"qT{pl}"),
                        v=vpool.tile([128, tb_n, 2, dh + 1], BF16,
                                     tag="v_sb", name=f"v{pl}"))
                return P[pl]

            def proj_chain(pl, which, t2):
                """One projection chain: 16 matmuls covering seq chunks
                (2*t2, 2*t2+1); consecutive matmuls share the stationary
                w[:, c] so only every other one pays LDWEIGHTS.  Yields once
                per PE op so the caller can weave."""
                wsl = slice(pl * 128, (pl + 1) * 128)
                pair = _alloc_pair(pl)
                dst, w_sb, bj = {
                    "k": (pair["kT"], wk_all, 1),
                    "q": (pair["qT"], wq_all, 0),
                    "v": (None, wv_all, 2),
                }[which]
                pss = [ps_pt.tile([128, 512], F32, tag="pt", name=f"ps{half}")
                       for half in range(2)]
                for c in range(kc):
                    for half in range(2):
                        t = 2 * t2 + half
                        nc.tensor.matmul(
                            pss[half], w_sb[:, c, wsl],
                            xT_sb[:, c, t * 512:(t + 1) * 512],
                            start=(c == 0), stop=(c == kc - 1))
                        yield
                for half in range(2):
                    t = 2 * t2 + half
                    if dst is not None:
                        nc.vector.tensor_scalar_add(
                            out=dst[:, t * 512:(t + 1) * 512], in0=pss[half],
                            scalar1=biases[:, bj, pl:pl + 1])
                    else:
                        # v path: bias into a staging tile, then 4
                        # transposes per psum tile -> [t, dv] blocks
                        vstg = vstage_pool.tile([128, 512], F32, tag="vstg")
                        nc.vector.tensor_scalar_add(
                            out=vstg, in0=pss[half],
                            scalar1=biases[:, 2, pl:pl + 1])
                        ptr = ps_pt.tile([128, 512], F32, tag="pt",
                                         name="ptr")
                        for j in range(4):
                            nc.tensor.transpose(
                                ptr[:, j * 128:(j + 1) * 128],
                                vstg[:, j * 128:(j + 1) * 128], ident)
                            yield
                        nc.vector.tensor_copy(
                            out=pair["v"][:, t * 4:(t + 1) * 4, :, 0:dh],
                            in_=ptr.rearrange("p (j h d) -> p j h d",
                                              j=4, h=2))

            def proj_main(pl):
                """Everything pair pl's attention needs from the start: full
                kT, first-half qT, full v (plus the denominator ones).
                Chain order matches DMA arrival: the t2=0 chains only need
                wk/wq/wv + xT seq-chunks 0-1; the t2=1 chains are gated on
                the last-shipped xT chunks 2-3 and would stall the in-order
                PE if emitted earlier."""
                for which, t2 in (("k", 0), ("q", 0), ("v", 0),
                                  ("k", 1), ("v", 1)):
                    yield from proj_chain(pl, which, t2)
                nc.vector.tensor_copy(
                    out=P[pl]["v"][:, :, :, dh:dh + 1],
                    in_=ones_col.to_broadcast([128, tb_n, 2, 1]))

# NOTE: weaving the k/v t2=1 chains into the pair's own attention loop
# races on hardware: the PE's 64-deep reorder window pulls the
# attention matmuls' LDWEIGHTS (kT / v_sb stationary reads) ahead of
# in-flight matmuls, crossing the DVE writes that produce those tiles
# (~40 instructions apart at best -- under the 64-instruction window).
# Manifested as nondeterministic NaN on pair 0.  Only the late-q chain
# (a MOVING operand, guarded by the matmul's own wait) may weave.

            def proj_late(pl):
                """Second-half qT -- only needed from sc2, so it can weave
                into pair pl's own attention."""
                yield from proj_chain(pl, "q", 1)

            def ln_emit(sbs):
                """Residual + LayerNorm for scrambled-row blocks sbs (all
                DVE).  Yields between the heavy DVE ops so the caller can
                weave them between DVE softmax-exp tiles -- the DVE is
                in-order, so an unbroken ~5us chain would stall the attention
                pipeline's et production.  Processing the blocks as one batch
                shares the rsqrt Newton chain (a dozen tiny [128,n] ops
                instead of per-block).  Output DMAs split across two queues
                so the final store isn't single-queue-bound."""
                n = len(sbs)
                xzs, mvs = {}, {}
                for i, sb in enumerate(sbs):
                    xz = lnw.tile([mrows, dim], BF16, tag="xz",
                                  name=f"xz{i}")
                    nc.vector.tensor_add(xz, xt4[:, sb, :], z_all[:, sb, :])
                    xzs[sb] = xz
                    yield
                    st = lnw.tile([mrows, 2, 6], F32, tag="st")
                    xz_g = xz.rearrange("p (g d) -> p g d", g=2)
                    for g in range(2):
                        nc.vector.bn_stats(out=st[:, g, :], in_=xz_g[:, g, :])
                        yield
                    mv = lnw.tile([mrows, 2], F32, tag="mv", name=f"mv{i}")
                    nc.vector.bn_aggr(out=mv, in_=st)
                    mvs[sb] = mv
                # rstd = (var+eps)^-0.5 entirely on DVE (quake init + 2
                # Newton steps on a [128,n] tile) -- keeps ACT's table set
                # pinned to exp_and_others so attention exps never reload
                # tables.  2 steps: 3.4% -> 0.18% -> 5e-6 rel err.
                veps = lnw.tile([mrows, n], F32, tag="veps")
                for i, sb in enumerate(sbs):
                    nc.vector.tensor_scalar_add(out=veps[:, i:i + 1],
                                                in0=mvs[sb][:, 1:2],
                                                scalar1=LN_EPS)
                I32 = mybir.dt.int32
                y = lnw.tile([mrows, n], F32, tag="rsq_y")
                # y0 bits = 0x5f3759df - (bits(veps) >> 1): shr (bitwise),
                # then negate+add (both arith; op0/op1 must share a class)
                nc.vector.tensor_scalar(
                    out=y.bitcast(I32), in0=veps.bitcast(I32),
                    scalar1=1, scalar2=None,
                    op0=mybir.AluOpType.logical_shift_right)
                nc.vector.tensor_scalar(
                    out=y.bitcast(I32), in0=y.bitcast(I32),
                    scalar1=-1, scalar2=0x5F3759DF,
                    op0=mybir.AluOpType.mult, op1=mybir.AluOpType.add)
                t = lnw.tile([mrows, n], F32, tag="rsq_t")
                for _ in range(2):
                    nc.vector.tensor_mul(t, y, y)
                    nc.vector.tensor_mul(t, t, veps)
                    nc.vector.tensor_scalar(
                        out=t, in0=t, scalar1=-0.5, scalar2=1.5,
                        op0=mybir.AluOpType.mult, op1=mybir.AluOpType.add)
                    nc.vector.tensor_mul(y, y, t)
                yield
                for i, sb in enumerate(sbs):
                    # xn computed in column halves, each half's store issued
                    # as soon as it exists (sync + scalar HWDGE rings; the
                    # gpsimd SWDGE path costs ~2us fixed and drains late).
                    # Shortens the final store's exposure at kernel end.
                    xn = lnw.tile([mrows, dim], BF16, tag="xn")
                    rows = slice(sb * mrows, (sb + 1) * mrows)
                    half = dim // 2
                    for hi, q in ((0, nc.sync), (1, nc.scalar)):
                        cs = slice(hi * half, (hi + 1) * half)
                        nc.vector.tensor_scalar(
                            out=xn[:, cs], in0=xzs[sb][:, cs],
                            scalar1=mvs[sb][:, 0:1],
                            scalar2=y[:, i:i + 1],
                            op0=mybir.AluOpType.subtract,
                            op1=mybir.AluOpType.mult)
                        if not trivial_affine:
                            nc.vector.tensor_mul(xn[:, cs], xn[:, cs],
                                                 grep[:mrows, cs])
                            nc.vector.tensor_add(xn[:, cs], xn[:, cs],
                                                 brep[:mrows, cs])
                        q.dma_start(out=out[rows, cs], in_=xn[:, cs])
                        yield

            def ztrans_units(pl):
                """Scrambled-z transposes + LN for pair pl; yields per PE op.
                Four j-columns of [s,65] land in one psum tile so the
                normalize is one reciprocal + one broadcast-multiply.  Both
                LN chains are emitted only after ALL transposes: the DVE is
                in-order, so a ~7us LN chain emitted mid-stream would make
                later transpose-normalizes (and with them the shared psum
                transpose tiles the PE is waiting on) queue behind it."""
                def hl_units(hl):
                    sb = 2 * pl + hl
                    zth = ZT[(pl, hl)]
                    zin_all = zth.rearrange("p (m j) -> p j m", j=16)
                    for j4 in range(4):
                        # dh+2 inner: keeps each jj slice 4B-aligned in PSUM
                        # (bf16 [.., 65] slices would land at 130B offsets)
                        ptz = ps_pt.tile([128, 4, dh + 2], BF16, tag="pt",
                                         name="ptz")
                        for jj in range(4):
                            nc.tensor.transpose(
                                ptz[:mrows, jj, 0:dh + 1],
                                zin_all[:, j4 * 4 + jj, :],
                                ident_bf[0:dh + 1, 0:dh + 1])
                            yield
                        rcb = rpool.tile([mrows, 4, 1], BF16, tag="recipb")
                        with nc.allow_low_precision(
                                reason="bf16 reciprocal for z-normalize; "
                                       "z is stored bf16 anyway"):
                            nc.vector.reciprocal(rcb,
                                                 ptz[:mrows, :, dh:dh + 1])
                        nc.vector.tensor_tensor(
                            out=z_all.rearrange(
                                "p h (j d) -> p h j d", j=16)
                            [:, sb, j4 * 4:(j4 + 1) * 4, :],
                            in0=ptz[:mrows, :, 0:dh],
                            in1=rcb.to_broadcast([mrows, 4, dh]),
                            op=mybir.AluOpType.mult)
                # hl0's transposes+normalize; then hl0's LN (DVE) interleaved
                # with hl1's transposes (PE) so the tail's LN chain hides
                # behind PE work; hl1's LN last.
                yield from hl_units(0)
                for _ in itertools.chain.from_iterable(
                        itertools.zip_longest(hl_units(1),
                                              ln_emit([2 * pl]))):
                    yield
                yield from ln_emit([2 * pl + 1])

            def attention(pl, bg, weave_fn, dve_exp_fn=None):
                """Score->exp->PV loop for pair pl, weaving background PE ops
                from generator bg between tb blocks.  Software-pipelined by
                one step: scores+exp for step g+1 are emitted BEFORE PV of
                step g, so the exp (the pacing engine) is never queued behind
                PV matmuls or woven background work.  dve_exp_fn(g) picks
                which iterations compute exp on the DVE (schraudolph) instead
                of ACT, splitting the softmax-exp load across both engines."""
                qT, kT, v_sb = P[pl]["qT"], P[pl]["kT"], P[pl]["v"]
                ets = {}
                zps_by_sc = {}

                def scores_exp(g):
                    sc, tb = divmod(g, tb_n)
                    ssp = ps_sT.tile([128, 1024], F32, tag="sT")
                    for hl in range(2):
                        hsl = slice(64 * hl, 64 * hl + 64)
                        nc.tensor.matmul(
                            ssp[:, hl * 512:(hl + 1) * 512],
                            kT[hsl, tb * 128:(tb + 1) * 128],
                            qT[hsl, sc * 512:(sc + 1) * 512],
                            start=True, stop=True)
                    if dve_exp_fn is not None and dve_exp_fn(g):
                        eti = epool.tile([128, 1024], I16, tag="expT")
                        nc.vector.tensor_scalar(
                            out=eti, in0=ssp,
                            scalar1=SCHRA_A, scalar2=SCHRA_B,
                            op0=mybir.AluOpType.mult,
                            op1=mybir.AluOpType.add)
                        et = eti.bitcast(BF16)
                    else:
                        et = epool.tile([128, 1024], BF16, tag="expT")
                        nc.scalar.activation(out=et, in_=ssp, func=EXP,
                                             scale=0.125)
                    ets[g] = et

                scores_exp(0)
                for g in range(sc_n * tb_n):
                    sc, tb = divmod(g, tb_n)
                    if g + 1 < sc_n * tb_n:
                        scores_exp(g + 1)
                    if tb == 0:
                        zps_by_sc[sc] = [
                            ps_z.tile([dh + 1, 512], F32, tag="zacc",
                                      name=f"zacc_{pl}_{sc}_{hl}")
                            for hl in range(2)]
                    zps = zps_by_sc[sc]
                    et = ets.pop(g)
                    for hl in range(2):
                        nc.tensor.matmul(
                            zps[hl], v_sb[:, tb, hl, :],
                            et[:, hl * 512:(hl + 1) * 512],
                            start=(tb == 0), stop=(tb == tb_n - 1))
                    if tb == tb_n - 1:
                        # evacuate zps BEFORE this iteration's weave units:
                        # the weave's DVE work would otherwise queue ahead of
                        # these copies on the in-order DVE, and the next sc's
                        # first PV stalls on the PSUM WAR until they land.
                        # Each copy splits DVE/ACT so both halves drain in
                        # parallel (~0.4us instead of ~1.4us serial).
                        for hl in range(2):
                            if (pl, hl) not in ZT:
                                ZT[(pl, hl)] = ztpool.tile(
                                    [dh + 1, s_keys], BF16, tag="ztsb",
                                    name=f"zth_{pl}_{hl}")
                            zt_dst = ZT[(pl, hl)]
                            c0 = sc * 512
                            nc.vector.tensor_copy(
                                out=zt_dst[:, c0:c0 + 256],
                                in_=zps[hl][:, 0:256])
                            nc.scalar.copy(
                                out=zt_dst[:, c0 + 256:c0 + 512],
                                in_=zps[hl][:, 256:512])
                    for _ in range(weave_fn(sc, tb)):
                        if next(bg, None) is None:
                            break

            def _drain(gen):
                for _ in gen:
                    pass

            # pair 0's minimal startup projections (seq chunks 0-1 only);
            # k/v t2=1 chains weave into the first 8 attention iterations
            # (deadline: scores/PV for tb=8), late qT + all of pair 1's
            # projections follow at 3/tb.
            _drain(proj_main(0))
            bg1 = itertools.chain(proj_late(0), proj_main(1))
            attention(0, bg1,
                      lambda sc, tb: 0 if (sc == 0 and tb < 2) else 2,
                      dve_exp_fn=lambda g: g % 4 == 1)
            _drain(bg1)
            # pair 1 attention, weaving its late qT, then pair 0's
            # z-transposes + LN(0), LN(1)
            bg2 = itertools.chain(proj_late(1), ztrans_units(0))
            attention(1, bg2, lambda sc, tb: 2 if sc < 2 else 1,
                      dve_exp_fn=lambda g: g % 4 == 1)
            _drain(bg2)
            # tail: pair 1's z-transposes + LN(2), LN(3)
            _drain(ztrans_units(1))

    return nc


def _shard_inputs(embedded, Wq, bq, Wk, bk, Wv, bv, gamma, beta):
    """Host-side sharding: transposes / slices / casts / concatenation only."""
    embedded = np.asarray(embedded, dtype=np.float32)
    c = np.ascontiguousarray
    bf = ml_dtypes.bfloat16
    WqT = np.asarray(Wq, dtype=np.float32).T
    WkT = np.asarray(Wk, dtype=np.float32).T
    WvT = np.asarray(Wv, dtype=np.float32).T
    bq = np.asarray(bq, np.float32)
    bk = np.asarray(bk, np.float32)
    bv = np.asarray(bv, np.float32)
    gb = {
        "gamma": c(np.asarray(gamma, np.float32)),
        "beta": c(np.asarray(beta, np.float32)),
    }
    xT_by_batch = [c(embedded[b].T.astype(bf)) for b in range(B)]
    in_maps = []
    for core in range(N_CORES):
        b, r = core // CPB, core % CPB
        rows = slice(r * SS, (r + 1) * SS)
        cols = slice(r * HPC * DH, (r + 1) * HPC * DH)
        in_maps.append({
            "xT": xT_by_batch[b],
            "xrow": c(embedded[b, rows].astype(bf)),
            "WqTs": c(WqT[:, cols].astype(bf)),
            "WkTs": c(WkT[:, cols].astype(bf)),
            "WvTs": c(WvT[:, cols].astype(bf)),
            "bqs": c(bq[cols]), "bks": c(bk[cols]), "bvs": c(bv[cols]),
            **gb,
        })
    return in_maps


_BUILT = {}


def _get_nc(trivial_affine=True):
    key = ("nc", trivial_affine)
    if key not in _BUILT:
        nc = bacc.Bacc("TRN2", debug=False, target_bir_lowering=False)
        build_mha(nc, trivial_affine=trivial_affine)
        nc.compile()
        _BUILT[key] = nc
    return _BUILT[key]


def kernel(embedded, Wq, bq, Wk, bk, Wv, bv, gamma, beta, _trace=False):
    trivial_affine = bool(
        np.all(np.asarray(gamma, np.float32) == 1.0)
        and np.all(np.asarray(beta, np.float32) == 0.0))
    nc = _get_nc(trivial_affine)
    in_maps = _shard_inputs(embedded, Wq, bq, Wk, bk, Wv, bv, gamma, beta)
    res = bass_utils.run_bass_kernel_spmd(
        nc, in_maps, core_ids=list(range(N_CORES)), trace=_trace)
    outs = [np.asarray(r["out"], dtype=np.float32) for r in res.results]
    full = np.stack([
        np.concatenate(outs[b * CPB:(b + 1) * CPB], axis=0) for b in range(B)
    ])
    if _trace:
        kernel._last_results = res
    return full.astype(np.float32)



# revision 57
# speedup vs baseline: 1.0019x; 1.0019x over previous
"""Multi-head attention + residual + LayerNorm kernel for 8 TRN2 NeuronCores.

Reference computation (B=2, S=2048, DIM=1024, H=16, DH=64):
    q = x @ Wq.T + bq ; k = x @ Wk.T + bk ; v = x @ Wv.T + bv     (per batch)
    attn_h = softmax((q_h @ k_h.T) / sqrt(DH))
    z_init[b,h,s,d] = attn_h @ v_h
    z = z_init.reshape(B, S, H*DH)      # "faithful" reshape WITHOUT the
                                        # [B,H,S,DH]->[B,S,H,DH] transpose!
    out = LayerNorm(x + z) * gamma + beta

Sharding: core c owns batch c//4 and heads 4r..4r+3 (r = c%4), i.e. output
rows 512r..512r+512.  No collectives; host does transposes/slices/concat.

Schedule: the attention inner loop's softmax exp is split between the
ACT engine (hardware exp, ~1.15us per [128,1024] tile) and the DVE (a
one-instruction Schraudolph bit-trick exp at ~1.22us: int16(s*A+B)
bitcast as bf16; its ~3% element error mostly cancels under softmax
normalization -- measured end-to-end ~8e-3 vs the 2e-2 gate).  With the
DVE taking every 4th tile, the loop paces at the PE's ~1.2us/iteration
(scores pair row-tiled via tile_position, concurrent; PV 2x512; two
woven background matmuls), not ACT's 1.15us -- so everything else is
*woven into* the loop:
  - all attention matmul operands are bf16 (q/k/v/et): f32r scores ran
    at half PE rate (fp32_mode=HIGH, 490ns vs 215ns per 512 cols).
    fp8 DoubleRow projections were measured numerically out of budget
    (2.5e-2 with fp8 q/k).
  - ~32 dummy matmuls on a zeroed tile warm the HAM clock gate during
    the input-DMA window, so the projection chains run at 2.4 GHz.
  - pair 1's projections weave (2 per tb) inside pair 0's attention;
    each pair's second-half qT (a MOVING operand) weaves into its own
    attention.  k/v chains must NOT weave into their own pair's
    attention: the PE's 64-deep reorder window pulls the attention
    LDWEIGHTS (stationary kT/v_sb reads) ahead of in-flight matmuls,
    racing the DVE writes (nondeterministic NaN on hardware).
  - pair 0's scrambled-z transposes (bf16, psum-padded to 66 cols for
    4B alignment) + LayerNorm weave inside pair 1's attention; pair 1's
    run in the tail with hl1's transposes (PE) interleaved against
    hl0's LN chain (DVE).
  - LayerNorm: residual add + stats + normalize all in bf16 (2x/4x DVE
    modes); 1/sqrt(var+eps) via quake bit-trick + 2 Newton steps on a
    batched [128,n] tile, keeping Sqrt off ACT so the exp table set
    never reloads (~2.7us).  gamma/beta passes are compiled out when
    the inputs are the trivial 1/0 (host-checked).
  - output is bf16 (cast to f32 on host); each LN block's store splits
    across the two HWDGE queues (sync + scalar).
Per-core dataflow: scores are computed transposed sT[t,s]=k.q so
softmax's reduction lands on the PV contraction (ones column appended
to v accumulates the denominator), zT blocks are PE-transposed into the
scrambled layout, LN runs rows-on-partitions.
"""

import itertools

import numpy as np
import ml_dtypes

import concourse.bass as bass
import concourse.bacc as bacc
import concourse.mybir as mybir
import concourse.tile as tile
from concourse import bass_utils
from concourse.masks import make_identity

F32 = mybir.dt.float32
F32R = mybir.dt.float32r
BF16 = mybir.dt.bfloat16
I32 = mybir.dt.int32

I16 = mybir.dt.int16

# Schraudolph exp on the DVE: exp(s/8) ~= bitcast_bf16(int16(s * SCHRA_A +
# SCHRA_B)).  A = 0.125 * 2^7/ln2 (score scale folded in); B = 127*2^7 - C
# with C tuned for min max-rel-error (~3%, which softmax normalization
# mostly cancels -- measured end-to-end rel err ~4e-3 at 100% schraudolph).
# One DVE tensor_scalar (f32 PSUM read, int16 convert-on-write) per tile;
# the result bitcasts to a bf16 exp estimate consumed by the PV matmul.
SCHRA_A = 16.0 / 0.6931471805599453
SCHRA_B = 16249.0

B, S, DIM, H = 2, 2048, 1024, 16
DH = DIM // H  # 64
N_CORES = 8
CPB = N_CORES // B   # cores per batch = 4
HPC = H // CPB       # heads per core = 4
SS = S // CPB        # output rows per core = 512
LN_EPS = 1e-5


def build_mha(nc: bass.Bass, trivial_affine: bool = True):
    s_keys, dim, hpc, dh = S, DIM, HPC, DH
    n_pairs = hpc // 2       # head pairs per core (2)
    kc = dim // 128          # contraction chunks (8)
    tb_n = s_keys // 128     # key blocks (16)
    sc_n = s_keys // 512     # 512-wide query chunks (4)
    mrows = s_keys // 16     # scrambled rows per head (128)
    ss_out = hpc * mrows     # output rows per core (512)
    wcols = hpc * dh         # per-core projection output cols (256)

    xT = nc.dram_tensor("xT", [dim, s_keys], BF16, kind="ExternalInput").ap()
    xrow = nc.dram_tensor("xrow", [ss_out, dim], BF16, kind="ExternalInput").ap()
    WqTs = nc.dram_tensor("WqTs", [dim, wcols], BF16, kind="ExternalInput").ap()
    WkTs = nc.dram_tensor("WkTs", [dim, wcols], BF16, kind="ExternalInput").ap()
    WvTs = nc.dram_tensor("WvTs", [dim, wcols], BF16, kind="ExternalInput").ap()
    bqs = nc.dram_tensor("bqs", [wcols], F32, kind="ExternalInput").ap()
    bks = nc.dram_tensor("bks", [wcols], F32, kind="ExternalInput").ap()
    bvs = nc.dram_tensor("bvs", [wcols], F32, kind="ExternalInput").ap()
    gamma = nc.dram_tensor("gamma", [dim], F32, kind="ExternalInput").ap()
    beta = nc.dram_tensor("beta", [dim], F32, kind="ExternalInput").ap()
    out = nc.dram_tensor("out", [ss_out, dim], BF16, kind="ExternalOutput").ap()

    EXP = mybir.ActivationFunctionType.Exp

    with tile.TileContext(nc) as tc:
        with tc.tile_pool(name="singles", bufs=1) as singles, \
             tc.tile_pool(name="kpool", bufs=2) as kpool, \
             tc.tile_pool(name="qpool", bufs=2) as qpool, \
             tc.tile_pool(name="vstage", bufs=2) as vstage_pool, \
             tc.tile_pool(name="vpool", bufs=2) as vpool, \
             tc.tile_pool(name="epool", bufs=4) as epool, \
             tc.tile_pool(name="ztpool", bufs=4) as ztpool, \
             tc.tile_pool(name="rpool", bufs=4) as rpool, \
             tc.tile_pool(name="lnx", bufs=1) as lnx, \
             tc.tile_pool(name="lnw", bufs=2) as lnw, \
             tc.tile_pool(name="ps_pt", bufs=2, space="PSUM") as ps_pt, \
             tc.tile_pool(name="ps_sT", bufs=2, space="PSUM") as ps_sT, \
             tc.tile_pool(name="ps_z", bufs=2, space="PSUM") as ps_z:

            # ---- persistent tiles -------------------------------------------
            xT_sb = singles.tile([128, kc, s_keys], BF16)
            z_all = singles.tile([mrows, hpc, dim], BF16)
            ident = singles.tile([128, 128], F32)
            make_identity(nc, ident)
            ident_bf = singles.tile([128, 128], BF16)
            make_identity(nc, ident_bf)
            ones_col = singles.tile([128, 1], F32)
            nc.vector.memset(ones_col, 1.0)
            # HAM warmup fodder: ~32 dummy matmuls on a zeroed tile keep the
            # PE busy through the input-DMA window, so the HAM clock gate is
            # already 8/8 when the first real projection chain issues (cold
            # chains at 1.2 GHz cost ~10us otherwise).
            warm = singles.tile([128, 512], BF16)
            nc.gpsimd.memset(warm, 0.0)
            for _ in range(24):
                wps = ps_pt.tile([128, 512], F32, tag="pt", name="warmps")
                nc.tensor.matmul(wps, warm[:, 0:128], warm,
                                 start=True, stop=True)
            wq_all = singles.tile([128, kc, wcols], BF16)
            wk_all = singles.tile([128, kc, wcols], BF16)
            wv_all = singles.tile([128, kc, wcols], BF16)
            biases = singles.tile([128, 3, n_pairs], F32)
            grep = singles.tile([128, dim], F32)
            brep = singles.tile([128, dim], F32)

            # ---- DMAs in need-by order --------------------------------------
            # weights first (gate the first projection chains), full-width
            # rows (512B lines), then xT by seq-chunk (kT consumes it
            # t-chunk by t-chunk).
            def _w_dma(w_sb, wt, ch):
                cs = slice(ch * (kc // 2), (ch + 1) * (kc // 2))
                nc.sync.dma_start(
                    out=w_sb[:, cs, :],
                    in_=bass.AP(tensor=wt.tensor,
                                offset=ch * (kc // 2) * 128 * wcols,
                                ap=[[wcols, 128], [128 * wcols, kc // 2],
                                    [1, wcols]]))

            def _x_dma(t, ch):
                cs = slice(ch * (kc // 2), (ch + 1) * (kc // 2))
                nc.sync.dma_start(
                    out=xT_sb[:, cs, t * 512:(t + 1) * 512],
                    in_=bass.AP(
                        tensor=xT.tensor,
                        offset=ch * (kc // 2) * 128 * s_keys + t * 512,
                        ap=[[s_keys, 128], [128 * s_keys, kc // 2], [1, 512]]))

            # need-by order: the k chain is gated on wk + xT chunks 0-1, so
            # those ship first; wq only gates the q chain that follows it.
            for ch in range(2):
                _w_dma(wk_all, WkTs, ch)
            for t in range(2):
                for ch in range(2):
                    _x_dma(t, ch)
            for ch in range(2):
                _w_dma(wq_all, WqTs, ch)
            for ch in range(2):
                _w_dma(wv_all, WvTs, ch)
            for t in range(2, sc_n):
                for ch in range(2):
                    _x_dma(t, ch)
            for j, bt in enumerate((bqs, bks, bvs)):
                nc.scalar.dma_start(
                    out=biases[:, j, :],
                    in_=bass.AP(tensor=bt.tensor, offset=0,
                                ap=[[1, 128], [128, n_pairs]]))
            if not trivial_affine:
                nc.scalar.dma_start(
                    out=grep, in_=bass.AP(tensor=gamma.tensor, offset=0,
                                          ap=[[0, 128], [1, dim]]))
                nc.scalar.dma_start(
                    out=brep, in_=bass.AP(tensor=beta.tensor, offset=0,
                                          ap=[[0, 128], [1, dim]]))
            # residual rows prefetch (read only by LN, which runs woven)
            xt4 = lnx.tile([mrows, hpc, dim], BF16, tag="xt4")
            for sb2 in range(2):
                nc.sync.dma_start(
                    out=xt4[:, 2 * sb2:2 * sb2 + 2, :],
                    in_=bass.AP(tensor=xrow.tensor,
                                offset=2 * sb2 * mrows * dim,
                                ap=[[dim, 128], [mrows * dim, 2], [1, dim]]))

            P = {}   # pl -> dict(qT=, kT=, v=)
            ZT = {}  # (pl, hl) -> zT accumulator tile

            def _alloc_pair(pl):
                if pl not in P:
                    P[pl] = dict(
                        kT=kpool.tile([128, s_keys], BF16, tag="kT",
                                      name=f"kT{pl}"),
                        qT=qpool.tile([128, s_keys], BF16, tag="qT",
                                      name=f"qT{pl}"),
                        v=vpool.tile([128, tb_n, 2, dh + 1], BF16,
                                     tag="v_sb", name=f"v{pl}"))
                return P[pl]

            def proj_chain(pl, which, t2):
                """One projection chain: 16 matmuls covering seq chunks
                (2*t2, 2*t2+1); consecutive matmuls share the stationary
                w[:, c] so only every other one pays LDWEIGHTS.  Yields once
                per PE op so the caller can weave."""
                wsl = slice(pl * 128, (pl + 1) * 128)
                pair = _alloc_pair(pl)
                dst, w_sb, bj = {
                    "k": (pair["kT"], wk_all, 1),
                    "q": (pair["qT"], wq_all, 0),
                    "v": (None, wv_all, 2),
                }[which]
                pss = [ps_pt.tile([128, 512], F32, tag="pt", name=f"ps{half}")
                       for half in range(2)]
                for c in range(kc):
                    for half in range(2):
                        t = 2 * t2 + half
                        nc.tensor.matmul(
                            pss[half], w_sb[:, c, wsl],
                            xT_sb[:, c, t * 512:(t + 1) * 512],
                            start=(c == 0), stop=(c == kc - 1))
                        yield
                for half in range(2):
                    t = 2 * t2 + half
                    if dst is not None:
                        nc.vector.tensor_scalar_add(
                            out=dst[:, t * 512:(t + 1) * 512], in0=pss[half],
                            scalar1=biases[:, bj, pl:pl + 1])
                    else:
                        # v path: bias into a staging tile, then 4
                        # transposes per psum tile -> [t, dv] blocks
                        vstg = vstage_pool.tile([128, 512], F32, tag="vstg")
                        nc.vector.tensor_scalar_add(
                            out=vstg, in0=pss[half],
                            scalar1=biases[:, 2, pl:pl + 1])
                        ptr = ps_pt.tile([128, 512], F32, tag="pt",
                                         name="ptr")
                        for j in range(4):
                            nc.tensor.transpose(
                                ptr[:, j * 128:(j + 1) * 128],
                                vstg[:, j * 128:(j + 1) * 128], ident)
                            yield
                        nc.vector.tensor_copy(
                            out=pair["v"][:, t * 4:(t + 1) * 4, :, 0:dh],
                            in_=ptr.rearrange("p (j h d) -> p j h d",
                                              j=4, h=2))

            def proj_main(pl):
                """Everything pair pl's attention needs from the start: full
                kT, first-half qT, full v (plus the denominator ones).
                Chain order matches DMA arrival: the t2=0 chains only need
                wk/wq/wv + xT seq-chunks 0-1; the t2=1 chains are gated on
                the last-shipped xT chunks 2-3 and would stall the in-order
                PE if emitted earlier."""
                for which, t2 in (("k", 0), ("q", 0), ("v", 0),
                                  ("k", 1), ("v", 1)):
                    yield from proj_chain(pl, which, t2)
                nc.vector.tensor_copy(
                    out=P[pl]["v"][:, :, :, dh:dh + 1],
                    in_=ones_col.to_broadcast([128, tb_n, 2, 1]))

# NOTE: weaving the k/v t2=1 chains into the pair's own attention loop
# races on hardware: the PE's 64-deep reorder window pulls the
# attention matmuls' LDWEIGHTS (kT / v_sb stationary reads) ahead of
# in-flight matmuls, crossing the DVE writes that produce those tiles
# (~40 instructions apart at best -- under the 64-instruction window).
# Manifested as nondeterministic NaN on pair 0.  Only the late-q chain
# (a MOVING operand, guarded by the matmul's own wait) may weave.

            def proj_late(pl):
                """Second-half qT -- only needed from sc2, so it can weave
                into pair pl's own attention."""
                yield from proj_chain(pl, "q", 1)

            def ln_emit(sbs):
                """Residual + LayerNorm for scrambled-row blocks sbs (all
                DVE).  Yields between the heavy DVE ops so the caller can
                weave them between DVE softmax-exp tiles -- the DVE is
                in-order, so an unbroken ~5us chain would stall the attention
                pipeline's et production.  Processing the blocks as one batch
                shares the rsqrt Newton chain (a dozen tiny [128,n] ops
                instead of per-block).  Output DMAs split across two queues
                so the final store isn't single-queue-bound."""
                n = len(sbs)
                xzs, mvs = {}, {}
                for i, sb in enumerate(sbs):
                    xz = lnw.tile([mrows, dim], BF16, tag="xz",
                                  name=f"xz{i}")
                    nc.vector.tensor_add(xz, xt4[:, sb, :], z_all[:, sb, :])
                    xzs[sb] = xz
                    yield
                    st = lnw.tile([mrows, 2, 6], F32, tag="st")
                    xz_g = xz.rearrange("p (g d) -> p g d", g=2)
                    for g in range(2):
                        nc.vector.bn_stats(out=st[:, g, :], in_=xz_g[:, g, :])
                        yield
                    mv = lnw.tile([mrows, 2], F32, tag="mv", name=f"mv{i}")
                    nc.vector.bn_aggr(out=mv, in_=st)
                    mvs[sb] = mv
                # rstd = (var+eps)^-0.5 entirely on DVE (quake init + 2
                # Newton steps on a [128,n] tile) -- keeps ACT's table set
                # pinned to exp_and_others so attention exps never reload
                # tables.  2 steps: 3.4% -> 0.18% -> 5e-6 rel err.
                veps = lnw.tile([mrows, n], F32, tag="veps")
                for i, sb in enumerate(sbs):
                    nc.vector.tensor_scalar_add(out=veps[:, i:i + 1],
                                                in0=mvs[sb][:, 1:2],
                                                scalar1=LN_EPS)
                I32 = mybir.dt.int32
                y = lnw.tile([mrows, n], F32, tag="rsq_y")
                # y0 bits = 0x5f3759df - (bits(veps) >> 1): shr (bitwise),
                # then negate+add (both arith; op0/op1 must share a class)
                nc.vector.tensor_scalar(
                    out=y.bitcast(I32), in0=veps.bitcast(I32),
                    scalar1=1, scalar2=None,
                    op0=mybir.AluOpType.logical_shift_right)
                nc.vector.tensor_scalar(
                    out=y.bitcast(I32), in0=y.bitcast(I32),
                    scalar1=-1, scalar2=0x5F3759DF,
                    op0=mybir.AluOpType.mult, op1=mybir.AluOpType.add)
                t = lnw.tile([mrows, n], F32, tag="rsq_t")
                for _ in range(2):
                    nc.vector.tensor_mul(t, y, y)
                    nc.vector.tensor_mul(t, t, veps)
                    nc.vector.tensor_scalar(
                        out=t, in0=t, scalar1=-0.5, scalar2=1.5,
                        op0=mybir.AluOpType.mult, op1=mybir.AluOpType.add)
                    nc.vector.tensor_mul(y, y, t)
                yield
                for i, sb in enumerate(sbs):
                    # xn computed in column halves, each half's store issued
                    # as soon as it exists (sync + scalar HWDGE rings; the
                    # gpsimd SWDGE path costs ~2us fixed and drains late).
                    # Shortens the final store's exposure at kernel end.
                    xn = lnw.tile([mrows, dim], BF16, tag="xn")
                    rows = slice(sb * mrows, (sb + 1) * mrows)
                    half = dim // 2
                    for hi, q in ((0, nc.sync), (1, nc.scalar)):
                        cs = slice(hi * half, (hi + 1) * half)
                        nc.vector.tensor_scalar(
                            out=xn[:, cs], in0=xzs[sb][:, cs],
                            scalar1=mvs[sb][:, 0:1],
                            scalar2=y[:, i:i + 1],
                            op0=mybir.AluOpType.subtract,
                            op1=mybir.AluOpType.mult)
                        if not trivial_affine:
                            nc.vector.tensor_mul(xn[:, cs], xn[:, cs],
                                                 grep[:mrows, cs])
                            nc.vector.tensor_add(xn[:, cs], xn[:, cs],
                                                 brep[:mrows, cs])
                        q.dma_start(out=out[rows, cs], in_=xn[:, cs])
                        yield

            def ztrans_units(pl):
                """Scrambled-z transposes + LN for pair pl; yields per PE op.
                Four j-columns of [s,65] land in one psum tile so the
                normalize is one reciprocal + one broadcast-multiply.  Both
                LN chains are emitted only after ALL transposes: the DVE is
                in-order, so a ~7us LN chain emitted mid-stream would make
                later transpose-normalizes (and with them the shared psum
                transpose tiles the PE is waiting on) queue behind it."""
                def hl_units(hl):
                    sb = 2 * pl + hl
                    zth = ZT[(pl, hl)]
                    zin_all = zth.rearrange("p (m j) -> p j m", j=16)
                    for j4 in range(4):
                        # dh+2 inner: keeps each jj slice 4B-aligned in PSUM
                        # (bf16 [.., 65] slices would land at 130B offsets)
                        ptz = ps_pt.tile([128, 4, dh + 2], BF16, tag="pt",
                                         name="ptz")
                        for jj in range(4):
                            nc.tensor.transpose(
                                ptz[:mrows, jj, 0:dh + 1],
                                zin_all[:, j4 * 4 + jj, :],
                                ident_bf[0:dh + 1, 0:dh + 1])
                            yield
                        rcb = rpool.tile([mrows, 4, 1], BF16, tag="recipb")
                        with nc.allow_low_precision(
                                reason="bf16 reciprocal for z-normalize; "
                                       "z is stored bf16 anyway"):
                            nc.vector.reciprocal(rcb,
                                                 ptz[:mrows, :, dh:dh + 1])
                        nc.vector.tensor_tensor(
                            out=z_all.rearrange(
                                "p h (j d) -> p h j d", j=16)
                            [:, sb, j4 * 4:(j4 + 1) * 4, :],
                            in0=ptz[:mrows, :, 0:dh],
                            in1=rcb.to_broadcast([mrows, 4, dh]),
                            op=mybir.AluOpType.mult)
                # hl0's transposes+normalize; then hl0's LN (DVE) interleaved
                # with hl1's transposes (PE) so the tail's LN chain hides
                # behind PE work; hl1's LN last.
                yield from hl_units(0)
                for _ in itertools.chain.from_iterable(
                        itertools.zip_longest(hl_units(1),
                                              ln_emit([2 * pl]))):
                    yield
                yield from ln_emit([2 * pl + 1])

            def attention(pl, bg, weave_fn, dve_exp_fn=None):
                """Score->exp->PV loop for pair pl, weaving background PE ops
                from generator bg between tb blocks.  Software-pipelined by
                one step: scores+exp for step g+1 are emitted BEFORE PV of
                step g, so the exp (the pacing engine) is never queued behind
                PV matmuls or woven background work.  dve_exp_fn(g) picks
                which iterations compute exp on the DVE (schraudolph) instead
                of ACT, splitting the softmax-exp load across both engines."""
                qT, kT, v_sb = P[pl]["qT"], P[pl]["kT"], P[pl]["v"]
                ets = {}
                zps_by_sc = {}

                def scores_exp(g):
                    sc, tb = divmod(g, tb_n)
                    ssp = ps_sT.tile([128, 1024], F32, tag="sT")
                    for hl in range(2):
                        hsl = slice(64 * hl, 64 * hl + 64)
                        nc.tensor.matmul(
                            ssp[:, hl * 512:(hl + 1) * 512],
                            kT[hsl, tb * 128:(tb + 1) * 128],
                            qT[hsl, sc * 512:(sc + 1) * 512],
                            start=True, stop=True)
                    if dve_exp_fn is not None and dve_exp_fn(g):
                        eti = epool.tile([128, 1024], I16, tag="expT")
                        nc.vector.tensor_scalar(
                            out=eti, in0=ssp,
                            scalar1=SCHRA_A, scalar2=SCHRA_B,
                            op0=mybir.AluOpType.mult,
                            op1=mybir.AluOpType.add)
                        et = eti.bitcast(BF16)
                    else:
                        et = epool.tile([128, 1024], BF16, tag="expT")
                        nc.scalar.activation(out=et, in_=ssp, func=EXP,
                                             scale=0.125)
                    ets[g] = et

                scores_exp(0)
                for g in range(sc_n * tb_n):
                    sc, tb = divmod(g, tb_n)
                    if g + 1 < sc_n * tb_n:
                        scores_exp(g + 1)
                    if tb == 0:
                        zps_by_sc[sc] = [
                            ps_z.tile([dh + 1, 512], F32, tag="zacc",
                                      name=f"zacc_{pl}_{sc}_{hl}")
                            for hl in range(2)]
                    zps = zps_by_sc[sc]
                    et = ets.pop(g)
                    for hl in range(2):
                        nc.tensor.matmul(
                            zps[hl], v_sb[:, tb, hl, :],
                            et[:, hl * 512:(hl + 1) * 512],
                            start=(tb == 0), stop=(tb == tb_n - 1))
                    if tb == tb_n - 1:
                        # evacuate zps BEFORE this iteration's weave units:
                        # the weave's DVE work would otherwise queue ahead of
                        # these copies on the in-order DVE, and the next sc's
                        # first PV stalls on the PSUM WAR until they land.
                        # Each copy splits DVE/ACT so both halves drain in
                        # parallel (~0.4us instead of ~1.4us serial).
                        for hl in range(2):
                            if (pl, hl) not in ZT:
                                ZT[(pl, hl)] = ztpool.tile(
                                    [dh + 1, s_keys], BF16, tag="ztsb",
                                    name=f"zth_{pl}_{hl}")
                            zt_dst = ZT[(pl, hl)]
                            c0 = sc * 512
                            nc.vector.tensor_copy(
                                out=zt_dst[:, c0:c0 + 256],
                                in_=zps[hl][:, 0:256])
                            nc.scalar.copy(
                                out=zt_dst[:, c0 + 256:c0 + 512],
                                in_=zps[hl][:, 256:512])
                    for _ in range(weave_fn(sc, tb)):
                        if next(bg, None) is None:
                            break

            def _drain(gen):
                for _ in gen:
                    pass

            # pair 0's minimal startup projections (seq chunks 0-1 only);
            # k/v t2=1 chains weave into the first 8 attention iterations
            # (deadline: scores/PV for tb=8), late qT + all of pair 1's
            # projections follow at 3/tb.
            _drain(proj_main(0))
            bg1 = itertools.chain(proj_late(0), proj_main(1))
            attention(0, bg1,
                      lambda sc, tb: 0 if (sc == 0 and tb < 2) else 2,
                      dve_exp_fn=lambda g: g % 4 == 1)
            _drain(bg1)
            # pair 1 attention, weaving its late qT, then pair 0's
            # z-transposes + LN(0), LN(1)
            bg2 = itertools.chain(proj_late(1), ztrans_units(0))
            # after bg2 exhausts (~g44) the DVE is idle and ACT would
            # pace the last iterations alone -- go 50/50 there
            attention(1, bg2, lambda sc, tb: 2 if sc < 2 else 1,
                      dve_exp_fn=lambda g: g % 4 == 1 or
                      (g >= 48 and g % 4 == 3))
            _drain(bg2)
            # tail: pair 1's z-transposes + LN(2), LN(3)
            _drain(ztrans_units(1))

    return nc


def _shard_inputs(embedded, Wq, bq, Wk, bk, Wv, bv, gamma, beta):
    """Host-side sharding: transposes / slices / casts / concatenation only."""
    embedded = np.asarray(embedded, dtype=np.float32)
    c = np.ascontiguousarray
    bf = ml_dtypes.bfloat16
    WqT = np.asarray(Wq, dtype=np.float32).T
    WkT = np.asarray(Wk, dtype=np.float32).T
    WvT = np.asarray(Wv, dtype=np.float32).T
    bq = np.asarray(bq, np.float32)
    bk = np.asarray(bk, np.float32)
    bv = np.asarray(bv, np.float32)
    gb = {
        "gamma": c(np.asarray(gamma, np.float32)),
        "beta": c(np.asarray(beta, np.float32)),
    }
    xT_by_batch = [c(embedded[b].T.astype(bf)) for b in range(B)]
    in_maps = []
    for core in range(N_CORES):
        b, r = core // CPB, core % CPB
        rows = slice(r * SS, (r + 1) * SS)
        cols = slice(r * HPC * DH, (r + 1) * HPC * DH)
        in_maps.append({
            "xT": xT_by_batch[b],
            "xrow": c(embedded[b, rows].astype(bf)),
            "WqTs": c(WqT[:, cols].astype(bf)),
            "WkTs": c(WkT[:, cols].astype(bf)),
            "WvTs": c(WvT[:, cols].astype(bf)),
            "bqs": c(bq[cols]), "bks": c(bk[cols]), "bvs": c(bv[cols]),
            **gb,
        })
    return in_maps


_BUILT = {}


def _get_nc(trivial_affine=True):
    key = ("nc", trivial_affine)
    if key not in _BUILT:
        nc = bacc.Bacc("TRN2", debug=False, target_bir_lowering=False)
        build_mha(nc, trivial_affine=trivial_affine)
        nc.compile()
        _BUILT[key] = nc
    return _BUILT[key]


def kernel(embedded, Wq, bq, Wk, bk, Wv, bv, gamma, beta, _trace=False):
    trivial_affine = bool(
        np.all(np.asarray(gamma, np.float32) == 1.0)
        and np.all(np.asarray(beta, np.float32) == 0.0))
    nc = _get_nc(trivial_affine)
    in_maps = _shard_inputs(embedded, Wq, bq, Wk, bk, Wv, bv, gamma, beta)
    res = bass_utils.run_bass_kernel_spmd(
        nc, in_maps, core_ids=list(range(N_CORES)), trace=_trace)
    outs = [np.asarray(r["out"], dtype=np.float32) for r in res.results]
    full = np.stack([
        np.concatenate(outs[b * CPB:(b + 1) * CPB], axis=0) for b in range(B)
    ])
    if _trace:
        kernel._last_results = res
    return full.astype(np.float32)

